# revision 2
# baseline (speedup 1.0000x reference)
"""FBGCN layer on 8 Trainium2 NeuronCores.

Math (reference):
    Lhp = (d_inv @ lap) @ d_inv
    Hh  = Lhp @ relu(x @ W_high)
    Hl  = GCNConv(x, edge_index, W_conv, b_conv)   (PyG-style, self-loops, sym norm)
    out = aL * Hl + aH * Hh

Kernel strategy:
  * Reassociate the dense chain: Hh = d_inv @ (lap @ (d_inv @ R)), R = relu(x @ W_high).
    This turns 2 N^3 matmuls into 3 N^2*D matmuls (10x fewer FLOPs).
  * Row-shard d_inv/lap across the 8 cores ([512,4096] slices, fed transposed as lhsT),
    AllGather the [4096,256] intermediate after steps 1 and 2.
  * GCN scatter is folded into a dense normalized-adjacency matmul:
    Hl = A_full @ (x @ W_conv) + b, with A_full[dst,src] = aL*dis[dst]*dis[src]*mult(dst,src)
    plus aL*dis[v]^2 on the diagonal (self loops); dis = deg^-1/2. A_full is built on the
    host in O(E) from edge_index and fed row-sharded (transposed) like lap.
  * All matmuls in bf16 (fp32 PSUM accumulation); aH is folded into W_high, aL into A/b.
"""

import numpy as np
import ml_dtypes

import concourse.bass as bass
import concourse.mybir as mybir
import concourse.tile as tile
from concourse import bacc
from concourse.bass_utils import run_bass_kernel_spmd

N = 4096
D = 256
E = 131072
NCORES = 8
RPC = N // NCORES          # rows per core = 512
KC = N // 128              # contraction chunks = 32
MT = RPC // 128            # output row tiles per core = 4
P = 128

BF16 = mybir.dt.bfloat16
F32 = mybir.dt.float32
nbf16 = ml_dtypes.bfloat16


def build_program(repeat: int = 1):
    """Build the SPMD per-core program. Identical on all cores."""
    nc = bacc.Bacc(num_devices=NCORES)

    # ---- I/O ----
    xT = nc.declare_dram_parameter("xT", [D, N], BF16, isOutput=False)
    Wh = nc.declare_dram_parameter("Wh", [D, D], BF16, isOutput=False)
    Wc = nc.declare_dram_parameter("Wc", [D, D], BF16, isOutput=False)
    dT = nc.declare_dram_parameter("dT", [N, RPC], BF16, isOutput=False)
    lT = nc.declare_dram_parameter("lT", [N, RPC], BF16, isOutput=False)
    aT = nc.declare_dram_parameter("aT", [N, RPC], BF16, isOutput=False)
    bL = nc.declare_dram_parameter("bL", [P, D], F32, isOutput=False)
    out = nc.declare_dram_parameter("out", [RPC, D], F32, isOutput=True)

    # collective bounce buffers
    cc1_in = nc.dram_tensor("cc1_in", [RPC, D], BF16)
    cc1_out = nc.dram_tensor("cc1_out", [N, D], BF16, addr_space="Shared")
    cc2_in = nc.dram_tensor("cc2_in", [RPC, D], BF16)
    cc2_out = nc.dram_tensor("cc2_out", [N, D], BF16, addr_space="Shared")

    # DRAM views: [(kc p) m] -> [p kc m]
    dT_v = dT.rearrange("(kc p) m -> p kc m", p=P)
    lT_v = lT.rearrange("(kc p) m -> p kc m", p=P)
    aT_v = aT.rearrange("(kc p) m -> p kc m", p=P)
    xT_v = xT.rearrange("(kc p) m -> p kc m", p=P)       # kc = 2
    Wh_v = Wh.rearrange("(kc p) m -> p kc m", p=P)
    Wc_v = Wc.rearrange("(kc p) m -> p kc m", p=P)
    cc1_out_v = cc1_out.rearrange("(kc p) m -> p kc m", p=P)
    cc2_out_v = cc2_out.rearrange("(kc p) m -> p kc m", p=P)
    cc1_in_v = cc1_in.rearrange("(mt p) m -> p mt m", p=P)
    cc2_in_v = cc2_in.rearrange("(mt p) m -> p mt m", p=P)
    out_v = out.rearrange("(mt p) m -> p mt m", p=P)

    NCHUNK = 4  # DMA chunks for the big matrices

    with tile.TileContext(nc) as tc:
        with (
            tc.tile_pool(name="const", bufs=1) as cpool,
            tc.tile_pool(name="bigmat", bufs=1) as bigpool,
            tc.tile_pool(name="acts", bufs=1) as apool,
            tc.tile_pool(name="psum", bufs=4, space="PSUM") as pspool,
            tc.tile_pool(name="outp", bufs=2) as opool,
        ):
            for _ in range(repeat):
                # ---- small loads ----
                xT_sb = cpool.tile([P, 2, N], BF16, tag="xT")
                Wh_sb = cpool.tile([P, 2, D], BF16, tag="Wh")
                Wc_sb = cpool.tile([P, 2, D], BF16, tag="Wc")
                bL_sb = cpool.tile([P, D], F32, tag="bL")
                nc.sync.dma_start(out=Wh_sb[:], in_=Wh_v)
                nc.sync.dma_start(out=Wc_sb[:], in_=Wc_v)
                nc.sync.dma_start(out=bL_sb[:], in_=bL[:])
                nc.sync.dma_start(out=xT_sb[:], in_=xT_v)

                # ---- big matrix loads (chunked so compute can start early) ----
                d_sb = bigpool.tile([P, KC, RPC], BF16, tag="d")
                a_sb = bigpool.tile([P, KC, RPC], BF16, tag="a")
                l_sb = bigpool.tile([P, KC, RPC], BF16, tag="l")
                kk = KC // NCHUNK
                for c in range(NCHUNK):
                    s = slice(c * kk, (c + 1) * kk)
                    nc.sync.dma_start(out=d_sb[:, s, :], in_=dT_v[:, s, :])
                for c in range(NCHUNK):
                    s = slice(c * kk, (c + 1) * kk)
                    nc.sync.dma_start(out=a_sb[:, s, :], in_=aT_v[:, s, :])
                for c in range(NCHUNK):
                    s = slice(c * kk, (c + 1) * kk)
                    nc.sync.dma_start(out=l_sb[:, s, :], in_=lT_v[:, s, :])

                # ---- stage A: R = relu(x @ (aH*W_high)), xw = x @ W_conv (replicated) ----
                R_sb = apool.tile([P, KC, D], BF16, tag="R")
                xw_sb = apool.tile([P, KC, D], BF16, tag="xw")
                for m in range(KC):
                    ps = pspool.tile([P, D], F32, tag="ps")
                    for k in range(2):
                        nc.tensor.matmul(
                            out=ps[:],
                            lhsT=xT_sb[:, k, m * P:(m + 1) * P],
                            rhs=Wh_sb[:, k, :],
                            start=(k == 0),
                            stop=(k == 1),
                        )
                    nc.vector.tensor_scalar_max(R_sb[:, m, :], ps[:], 0.0)
                    ps2 = pspool.tile([P, D], F32, tag="ps")
                    for k in range(2):
                        nc.tensor.matmul(
                            out=ps2[:],
                            lhsT=xT_sb[:, k, m * P:(m + 1) * P],
                            rhs=Wc_sb[:, k, :],
                            start=(k == 0),
                            stop=(k == 1),
                        )
                    nc.vector.tensor_copy(xw_sb[:, m, :], ps2[:])

                # ---- stage B: P1_loc = d_inv[rows] @ R ; AllGather -> P1 ----
                for m in range(MT):
                    ps = pspool.tile([P, D], F32, tag="ps")
                    for k in range(KC):
                        nc.tensor.matmul(
                            out=ps[:],
                            lhsT=d_sb[:, k, m * P:(m + 1) * P],
                            rhs=R_sb[:, k, :],
                            start=(k == 0),
                            stop=(k == KC - 1),
                        )
                    p1b = opool.tile([P, D], BF16, tag="p1b")
                    nc.vector.tensor_copy(p1b[:], ps[:])
                    nc.sync.dma_start(out=cc1_in_v[:, m, :], in_=p1b[:])
                nc.gpsimd.collective_compute(
                    "AllGather",
                    mybir.AluOpType.bypass,
                    replica_groups=[list(range(NCORES))],
                    ins=[cc1_in[:]],
                    outs=[cc1_out[:]],
                )
                P1_sb = apool.tile([P, KC, D], BF16, tag="P1")
                for c in range(NCHUNK):
                    s = slice(c * kk, (c + 1) * kk)
                    nc.sync.dma_start(out=P1_sb[:, s, :], in_=cc1_out_v[:, s, :])

                # ---- stage C: Hl = A_full[rows] @ xw + b (overlaps AllGather) ----
                Hl_sb = opool.tile([P, MT, D], F32, tag="Hl")
                for m in range(MT):
                    ps = pspool.tile([P, D], F32, tag="ps")
                    for k in range(KC):
                        nc.tensor.matmul(
                            out=ps[:],
                            lhsT=a_sb[:, k, m * P:(m + 1) * P],
                            rhs=xw_sb[:, k, :],
                            start=(k == 0),
                            stop=(k == KC - 1),
                        )
                    nc.vector.tensor_add(Hl_sb[:, m, :], ps[:], bL_sb[:])

                # ---- stage D: P2_loc = lap[rows] @ P1 ; AllGather -> P2 ----
                for m in range(MT):
                    ps = pspool.tile([P, D], F32, tag="ps")
                    for k in range(KC):
                        nc.tensor.matmul(
                            out=ps[:],
                            lhsT=l_sb[:, k, m * P:(m + 1) * P],
                            rhs=P1_sb[:, k, :],
                            start=(k == 0),
                            stop=(k == KC - 1),
                        )
                    p2b = opool.tile([P, D], BF16, tag="p2b")
                    nc.vector.tensor_copy(p2b[:], ps[:])
                    nc.sync.dma_start(out=cc2_in_v[:, m, :], in_=p2b[:])
                nc.gpsimd.collective_compute(
                    "AllGather",
                    mybir.AluOpType.bypass,
                    replica_groups=[list(range(NCORES))],
                    ins=[cc2_in[:]],
                    outs=[cc2_out[:]],
                )
                P2_sb = apool.tile([P, KC, D], BF16, tag="P2")
                for c in range(NCHUNK):
                    s = slice(c * kk, (c + 1) * kk)
                    nc.sync.dma_start(out=P2_sb[:, s, :], in_=cc2_out_v[:, s, :])

                # ---- stage E: out = Hl + d_inv[rows] @ P2 ----
                for m in range(MT):
                    ps = pspool.tile([P, D], F32, tag="ps")
                    for k in range(KC):
                        nc.tensor.matmul(
                            out=ps[:],
                            lhsT=d_sb[:, k, m * P:(m + 1) * P],
                            rhs=P2_sb[:, k, :],
                            start=(k == 0),
                            stop=(k == KC - 1),
                        )
                    o_sb = opool.tile([P, D], F32, tag="osb")
                    nc.vector.tensor_add(o_sb[:], ps[:], Hl_sb[:, m, :])
                    nc.sync.dma_start(out=out_v[:, m, :], in_=o_sb[:])

    nc.finalize()
    return nc


def prep_inputs(x, edge_index, lap, d_inv, W_high, W_conv, b_conv, aL, aH):
    """Host-side sharding/layout: build per-core input maps."""
    x = np.asarray(x, dtype=np.float32)
    lap = np.asarray(lap, dtype=np.float32)
    d_inv = np.asarray(d_inv, dtype=np.float32)
    W_high = np.asarray(W_high, dtype=np.float32)
    W_conv = np.asarray(W_conv, dtype=np.float32)
    b_conv = np.asarray(b_conv, dtype=np.float32)
    aLs = float(np.asarray(aL).reshape(-1)[0])
    aHs = float(np.asarray(aH).reshape(-1)[0])
    src = np.asarray(edge_index[0], dtype=np.int64)
    dst = np.asarray(edge_index[1], dtype=np.int64)

    # symmetric GCN normalization (with self-loops) folded into a dense adjacency
    deg = np.bincount(dst, minlength=N).astype(np.float32) + 1.0
    dis = 1.0 / np.sqrt(deg)
    A_T = np.zeros((N, N), dtype=np.float32)           # A_T[src, dst]
    np.add.at(A_T, (src, dst), aLs * dis[src] * dis[dst])
    A_T[np.arange(N), np.arange(N)] += aLs * dis * dis

    xT = np.ascontiguousarray(x.T).astype(nbf16)
    Wh = (W_high * aHs).astype(nbf16)
    Wc = W_conv.astype(nbf16)
    bLb = np.broadcast_to(aLs * b_conv, (P, D)).astype(np.float32).copy()
    dT_full = np.ascontiguousarray(d_inv.T).astype(nbf16)
    lT_full = np.ascontiguousarray(lap.T).astype(nbf16)
    aT_full = A_T.astype(nbf16)

    in_maps = []
    for i in range(NCORES):
        sl = slice(i * RPC, (i + 1) * RPC)
        in_maps.append({
            "xT": xT,
            "Wh": Wh,
            "Wc": Wc,
            "dT": np.ascontiguousarray(dT_full[:, sl]),
            "lT": np.ascontiguousarray(lT_full[:, sl]),
            "aT": np.ascontiguousarray(aT_full[:, sl]),
            "bL": bLb,
        })
    return in_maps


def kernel(x, edge_index, lap, d_inv, W_high, W_conv, b_conv, aL, aH):
    in_maps = prep_inputs(x, edge_index, lap, d_inv, W_high, W_conv, b_conv, aL, aH)
    nc = build_program()
    res = run_bass_kernel_spmd(nc, in_maps, list(range(NCORES)))
    return np.concatenate([res.results[i]["out"] for i in range(NCORES)], axis=0)
